# revision 19
# baseline (speedup 1.0000x reference)
"""Class-balanced segmentation loss on 8 Trainium2 NeuronCores.

Math: with counts_c = #{p: t_p == c}, S = sum_p lse_p, T_c = sum_{t_p=c}
pred[c, p], and w_c = 0.001 / (1 - 0.999**counts_c) (0 for empty classes):

    loss = (sum_c w_c * (S_c - T_c)) / (sum_c w_c * counts_c)

The histogram over the integer target is computed on the host (np.bincount
while laying out/sharding the inputs); the weights w_c then multiply the
per-class DEVICE partials, so the device never needs a counts pass:

  - fast path (all pixels valid, all w_c equal -- always true for this
    data regime, where 0.999**counts underflows and every w_c == 0.001):
    numerator = w * (sum_p lse_p - sum_c T_c), denominator = w * N.
    sum_p lse_p rides the ln activation's accum_out for free.
  - general path (ignore_index pixels or unequal weights): a per-pixel
    weight map W_p = w_(t_p) * valid_p is prepared on the host, and the
    device computes sum_p W_p*lse_p with one extra tensor_tensor per chunk.

Device pass per core (one batch; pixels on partitions, [128, 19, F] chunks):
  exp (one ACT instr over all 19 classes) -> sumexp over classes (DVE
  tensor_tensor chain, optional gpsimd side-chain) -> ln with accum_out
  (ACT) -> per class one fused STT (t==c)*pred_c with accum_out -> DMA out
  the [128, 19*NCH] fp32 T partials + [128, NCH] S partials.
"""

import os

import numpy as np

NCLASS = 19
B, H, W = 8, 512, 512
NPIX = H * W          # 262144 pixels per batch
P = 128               # SBUF partitions
FW = NPIX // P        # 2048 free-dim elements per partition
NCORES = 8
BETA = 1.0 - 0.001

F = int(os.environ.get("CHUNK_F", "1024"))  # free-dim chunk size
NCH = FW // F                               # chunks per batch
GPS_SUMEXP = int(os.environ.get("GPS_SUMEXP", "0"))  # adds on gpsimd
SUMEXP_TREE = int(os.environ.get("SUMEXP_TREE", "0"))
SKIP_STT = int(os.environ.get("SKIP_STT", "0"))      # attribution probes
SKIP_SUMEXP = int(os.environ.get("SKIP_SUMEXP", "0"))
SKIP_EXP = int(os.environ.get("SKIP_EXP", "0"))
# T-side select implementation:
#   stt      = scalar_tensor_tensor fused (1x-only uop, slow)
#   ts_ttr   = tensor_scalar mask @4x + tensor_tensor_reduce accum
#   ts_tt_pe = tensor_scalar mask @4x + tensor_tensor mult @2x + PE reduce
#   gps_tt_pe= gpsimd mask + tensor_tensor mult @2x + PE reduce
TSEL = os.environ.get("TSEL", "ts_tt_pe")
MMW = int(os.environ.get("MMW", "512"))

_COMPILED = {}


def _np_bf16():
    import ml_dtypes

    return ml_dtypes.bfloat16


def _patch_tile_drain():
    """walrus in this container rejects >1 sem-wait on one instruction
    ("Too many sync wait commands"); the tile-exit Drain carries one wait
    per logical processor. Split them into single-wait NOPs."""
    import bass_rust
    import concourse.tile as tile

    if getattr(tile.TileContext, "_drain_patched", False):
        return

    def _drain_and_barrier(self, tick_clock, wait_clock):
        from concourse.tile import ScopedClock

        probe = self.nc.sync.nop(nofuse=True)
        wait_clock.add_sem_waits(
            probe.ins, ScopedClock({None: tick_clock.global_clock})
        )
        si = probe.ins.sync_info
        waits = list(si.on_wait) if si else []
        if si:
            si.on_wait = waits[:1]
        for i in range(1, len(waits)):
            n = self.nc.sync.nop(nofuse=True)
            n.ins.sync_info = bass_rust.SyncInfo(
                on_wait=waits[i : i + 1], on_update=[]
            )
        self.nc.sync.drain()
        self.nc.all_engine_barrier()
        assert self.sems is not None
        popped = self.nc._tile_sem_poison_stack.pop()
        assert popped is self._sem_poison
        self.nc.clear_and_free_semaphores(list(self.sems.allocated().values()))
        self.nc.all_engine_barrier()

    tile.TileContext._drain_and_barrier = _drain_and_barrier
    tile.TileContext._drain_patched = True


def _split_excess_waits(nc, maxw=1):
    """Post-pass: any instruction carrying more than `maxw` sem-waits gets
    the extras moved onto same-engine NOPs inserted right before it (the
    engine executes in order, so semantics are identical)."""
    import bass_rust

    for blk in nc.m.functions[0].blocks:
        insts = list(blk.instructions)
        out = []
        changed = False
        for inst in insts:
            si = inst.sync_info
            if si is not None and si.on_wait and len(si.on_wait) > maxw:
                waits = list(si.on_wait)
                si.on_wait = waits[:maxw]
                extra = waits[maxw:]
                eng = nc.engines[inst.engine]
                for i in range(0, len(extra), maxw):
                    n = eng.nop(nofuse=True)
                    cur = nc.cur_bb.bb
                    cur_insts = list(cur.instructions)
                    assert cur_insts[-1].name == n.ins.name
                    cur.instructions = cur_insts[:-1]
                    n.ins.sync_info = bass_rust.SyncInfo(
                        on_wait=extra[i : i + maxw], on_update=[]
                    )
                    out.append(n.ins)
                changed = True
            out.append(inst)
        if changed:
            blk.instructions = out


def build_nc(reps: int = 1, general: bool = False):
    """Per-core Bass program (SPMD over 8 cores, one batch each).

    Inputs: pred [P, NCH, NCLASS, F] bf16 (class-major chunks, host
    pre-transposed), targ [P, FW] bf16 (class ids as floats; invalid
    pixels remapped to -5 so no is_equal matches). General variant adds
    wmap [P, FW] bf16 (per-pixel class weight, 0 for invalid).

    Outputs: out_t [P, NCH*NCLASS] fp32 (T partials, col k*19+c),
    out_s [P, NCH] fp32 (sum of lse per chunk; general: W-weighted).
    """
    from contextlib import ExitStack

    import concourse.bass as bass
    import concourse.tile as tile
    from concourse import mybir

    _patch_tile_drain()

    io_dt = mybir.dt.bfloat16
    nc = bass.Bass()
    pred = nc.declare_dram_parameter(
        "pred", [P, NCH, NCLASS, F], io_dt, isOutput=False
    )
    targ = nc.declare_dram_parameter("targ", [P, FW], io_dt, isOutput=False)
    if general:
        wmap = nc.declare_dram_parameter(
            "wmap", [P, FW], io_dt, isOutput=False
        )
    use_pe_out = TSEL in ("ts_tt_pe", "gps_tt_pe")
    if use_pe_out:
        out_tr = nc.declare_dram_parameter(
            "out_tr", [NCLASS, 1], mybir.dt.float32, isOutput=True
        )
    else:
        out_t = nc.declare_dram_parameter(
            "out_t", [P, NCH * NCLASS], mybir.dt.float32, isOutput=True
        )
    out_s = nc.declare_dram_parameter(
        "out_s", [P, NCH], mybir.dt.float32, isOutput=True
    )

    use_pe = TSEL in ("ts_tt_pe", "gps_tt_pe")

    with tile.TileContext(nc) as tc:
        with ExitStack() as ctx:
            io = ctx.enter_context(tc.tile_pool(name="io", bufs=2))
            work = ctx.enter_context(tc.tile_pool(name="work", bufs=2))
            pp = ctx.enter_context(tc.tile_pool(name="pp", bufs=3))
            acc = ctx.enter_context(tc.tile_pool(name="acc", bufs=1))
            if use_pe:
                psp = ctx.enter_context(
                    tc.tile_pool(name="psp", bufs=1, space="PSUM")
                )

            t_acc = acc.tile([P, NCH * NCLASS], mybir.dt.float32)
            nc.vector.memset(t_acc[:, :], 0.0)
            s_acc = acc.tile([P, NCH], mybir.dt.float32)
            nc.vector.memset(s_acc[:, :], 0.0)

            if use_pe:
                # place[:, c, :]: [128, NCLASS] one-hot col c -- stationary
                # operand routing a partition-sum into PSUM row c
                place = acc.tile([P, NCLASS, NCLASS], io_dt)
                nc.vector.memset(place[:, :, :], 0.0)
                for c in range(NCLASS):
                    nc.vector.memset(place[:, c, c : c + 1], 1.0)
                ps_t = psp.tile([NCLASS, F], mybir.dt.float32, tag="psT")
                t_red = acc.tile([NCLASS, 1], mybir.dt.float32)
                nc.vector.memset(t_red[:, :], 0.0)

            def _body():
                # Software pipeline, explicit per-engine program order:
                #   ACT: exp(0), exp(1), ..., ln(0), ln(1), ...
                #   DVE: STT(0), STT(1), ..., sumexp(0), sumexp(1), ...
                # so exp(k+1) never waits behind ln(k), and the STTs (which
                # only need the DMA) fill the DVE while ACT runs exp.
                p_tiles, t_tiles, e_tiles, w_tiles = [], [], [], []
                for k in range(NCH):
                    p_tile = io.tile([P, NCLASS, F], io_dt, tag="p")
                    nc.sync.dma_start(out=p_tile[:, :, :], in_=pred[:, k, :, :])
                    t_tile = io.tile([P, F], io_dt, tag="t")
                    nc.sync.dma_start(
                        out=t_tile[:], in_=targ[:, k * F : (k + 1) * F]
                    )
                    if general:
                        w_tile = io.tile([P, F], io_dt, tag="w")
                        nc.sync.dma_start(
                            out=w_tile[:], in_=wmap[:, k * F : (k + 1) * F]
                        )
                        w_tiles.append(w_tile)
                    p_tiles.append(p_tile)
                    t_tiles.append(t_tile)

                    # exp of all classes in one ACT instruction
                    if SKIP_EXP:
                        e_tiles.append(p_tile)  # probe: no ACT exp at all
                    else:
                        e_tile = work.tile([P, NCLASS, F], io_dt, tag="e")
                        nc.scalar.activation(
                            out=e_tile[:, :, :],
                            in_=p_tile[:, :, :],
                            func=mybir.ActivationFunctionType.Exp,
                        )
                        e_tiles.append(e_tile)

                    # T partials: per class, (t==c)*pred_c summed over pixels.
                    # scalar_tensor_tensor only has a 1x uop, so the fast
                    # variants split it: mask via tensor_scalar (4x) or
                    # gpsimd, product via tensor_tensor (2x), reduction via
                    # idle-PE matmul against the one-hot `place` stationary.
                    for c in range(NCLASS if not SKIP_STT else 0):
                        col = k * NCLASS + c
                        if TSEL == "stt":
                            prod = pp.tile([P, F], io_dt, tag="prod")
                            nc.vector.scalar_tensor_tensor(
                                out=prod[:],
                                in0=t_tile[:],
                                scalar=float(c),
                                in1=p_tile[:, c, :],
                                op0=mybir.AluOpType.is_equal,
                                op1=mybir.AluOpType.mult,
                                accum_out=t_acc[:, col : col + 1],
                            )
                            continue
                        mask = pp.tile([P, F], io_dt, tag="mask")
                        if TSEL == "gps_tt_pe":
                            nc.gpsimd.tensor_scalar(
                                out=mask[:],
                                in0=t_tile[:],
                                scalar1=float(c),
                                scalar2=None,
                                op0=mybir.AluOpType.is_equal,
                            )
                        else:
                            nc.vector.tensor_scalar(
                                out=mask[:],
                                in0=t_tile[:],
                                scalar1=float(c),
                                scalar2=None,
                                op0=mybir.AluOpType.is_equal,
                            )
                        if TSEL == "ts_ttr":
                            prod = pp.tile([P, F], io_dt, tag="prod")
                            nc.vector.tensor_tensor_reduce(
                                out=prod[:],
                                in0=mask[:],
                                in1=p_tile[:, c, :],
                                scale=1.0,
                                scalar=0.0,
                                op0=mybir.AluOpType.mult,
                                op1=mybir.AluOpType.add,
                                accum_out=t_acc[:, col : col + 1],
                            )
                        else:
                            prod = pp.tile([P, F], io_dt, tag="prod")
                            nc.vector.tensor_tensor(
                                prod[:], mask[:], p_tile[:, c, :],
                                mybir.AluOpType.mult,
                            )
                            for s in range(F // MMW):
                                nc.tensor.matmul(
                                    ps_t[:, s * MMW : (s + 1) * MMW],
                                    place[:, c, :],
                                    prod[:, s * MMW : (s + 1) * MMW],
                                    start=(k == 0 and c == 0),
                                    stop=(k == NCH - 1 and c == NCLASS - 1),
                                )

                # sumexp over classes, then ln (ACT) behind all exps
                sxs = []
                for k in range(NCH):
                    e_tile = e_tiles[k]
                    sx = work.tile([P, F], io_dt, tag="sx")
                    if SKIP_SUMEXP:
                        nc.vector.tensor_tensor(
                            sx[:], e_tile[:, 0, :], e_tile[:, 1, :],
                            mybir.AluOpType.add,
                        )
                    else:
                        nc.vector.tensor_tensor(
                            sx[:], e_tile[:, 0, :], e_tile[:, 1, :],
                            mybir.AluOpType.add,
                        )
                        for c in range(2, NCLASS):
                            nc.vector.tensor_tensor(
                                sx[:], sx[:], e_tile[:, c, :],
                                mybir.AluOpType.add,
                            )
                    sxs.append(sx)

                if use_pe and not SKIP_STT:
                    # drain PSUM: free-reduce [19, F] -> [19, 1]
                    junk = work.tile([NCLASS, F], io_dt, tag="junk")
                    nc.scalar.activation(
                        out=junk[:, :],
                        in_=ps_t[:, :],
                        func=mybir.ActivationFunctionType.Copy,
                        accum_out=t_red[:, 0:1],
                    )

                for k in range(NCH):
                    lse = work.tile([P, F], io_dt, tag="lse")
                    if general:
                        nc.scalar.activation(
                            out=lse[:],
                            in_=sxs[k][:],
                            func=mybir.ActivationFunctionType.Ln,
                        )
                        wl = pp.tile([P, F], io_dt, tag="wl")
                        nc.vector.tensor_tensor_reduce(
                            out=wl[:],
                            in0=w_tiles[k][:],
                            in1=lse[:],
                            scale=1.0,
                            scalar=0.0,
                            op0=mybir.AluOpType.mult,
                            op1=mybir.AluOpType.add,
                            accum_out=s_acc[:, k : k + 1],
                        )
                    else:
                        # sum_f lse rides the ln's accum_out for free
                        nc.scalar.activation(
                            out=lse[:],
                            in_=sxs[k][:],
                            func=mybir.ActivationFunctionType.Ln,
                            accum_out=s_acc[:, k : k + 1],
                        )

            if reps == 1:
                _body()
            else:
                with tc.For_i(0, reps, 1):
                    _body()

            if use_pe:
                nc.sync.dma_start(out=out_tr[:, :], in_=t_red[:, :])
            else:
                nc.sync.dma_start(out=out_t[:, :], in_=t_acc[:, :])
            nc.sync.dma_start(out=out_s[:, :], in_=s_acc[:, :])

    _split_excess_waits(nc, maxw=1)
    return nc


def get_nc(reps: int = 1, general: bool = False):
    key = (reps, general)
    if key not in _COMPILED:
        _COMPILED[key] = build_nc(reps, general)
    return _COMPILED[key]


def _class_weights(targ_np):
    """Histogram of the full target -> quantized class weights (bf16, so
    the device W map and the host denominator use identical values)."""
    valid = targ_np >= 0
    counts = np.bincount(
        targ_np[valid].ravel().astype(np.int64), minlength=NCLASS
    )[:NCLASS].astype(np.float64)
    with np.errstate(divide="ignore", over="ignore", under="ignore"):
        w = np.float32(1.0 - BETA) / (
            1.0 - np.float32(BETA) ** counts.astype(np.float32)
        )
    w = np.where(counts > 0, w, 0.0).astype(np.float32)
    wq = w.astype(_np_bf16()).astype(np.float64)
    return counts, wq, valid


def _shard_inputs(pred_np, targ_np, wmap=None, t_enc=None):
    dt = _np_bf16()
    if t_enc is None:
        t_enc = targ_np.astype(np.float32)
    in_maps = []
    for b in range(NCORES):
        # [19, 262144] -> [P, NCH, NCLASS, F]
        pb = pred_np[b].reshape(NCLASS, P, NCH, F).transpose(1, 2, 0, 3)
        m = {
            "pred": np.ascontiguousarray(pb).astype(dt),
            "targ": t_enc[b].reshape(P, FW).astype(dt),
        }
        if wmap is not None:
            m["wmap"] = wmap[b].reshape(P, FW).astype(dt)
        in_maps.append(m)
    return in_maps


def _run_device(pred_np, targ_np, reps: int = 1, in_maps=None, general=False):
    from concourse.bass_utils import run_bass_kernel_spmd

    nc = get_nc(reps, general)
    if in_maps is None:
        in_maps = _shard_inputs(pred_np, targ_np)
    res = run_bass_kernel_spmd(nc, in_maps, core_ids=list(range(NCORES)))
    return [res.results[i] for i in range(NCORES)]


def kernel(pred: np.ndarray, target: np.ndarray) -> np.ndarray:
    pred_np = np.asarray(pred, dtype=np.float32)
    targ_np = np.asarray(target)

    counts, wq, valid = _class_weights(targ_np)
    uniform = bool(valid.all()) and bool(
        np.all(wq == wq[0]) and wq[0] > 0
    )

    if uniform:
        in_maps = _shard_inputs(pred_np, targ_np)
        outs = _run_device(pred_np, targ_np, in_maps=in_maps, general=False)
        S = np.float64(0.0)
        T = np.zeros(NCLASS, np.float64)
        for r in outs:
            T += _t_partials(r)
            S += np.asarray(r["out_s"], np.float64).sum()
        num = wq[0] * (S - float(T.sum()))
        den = wq[0] * float(counts.sum())
        return np.array(np.float32(num / den))

    # general path: per-pixel weight map; invalid pixels get weight 0 and
    # a target code (-5) that matches no class
    wmap = (wq[np.clip(targ_np, 0, NCLASS - 1)] * valid).astype(np.float32)
    t_enc = np.where(valid, targ_np, -5).astype(np.float32)
    in_maps = _shard_inputs(pred_np, targ_np, wmap=wmap, t_enc=t_enc)
    outs = _run_device(pred_np, targ_np, in_maps=in_maps, general=True)
    SW = np.float64(0.0)
    T = np.zeros(NCLASS, np.float64)
    for r in outs:
        T += _t_partials(r)
        SW += np.asarray(r["out_s"], np.float64).sum()
    num = SW - float((wq * T).sum())
    den = float((wq * counts).sum())
    return np.array(np.float32(num / den))


def _t_partials(r):
    if "out_tr" in r:
        return np.asarray(r["out_tr"], np.float64)[:, 0]
    return (
        np.asarray(r["out_t"], np.float64).reshape(P, NCH, NCLASS).sum((0, 1))
    )


# revision 21
# speedup vs baseline: 9.5581x; 9.5581x over previous
"""Class-balanced segmentation loss on 8 Trainium2 NeuronCores.

Math: with counts_c = #{p: t_p == c}, S = sum_p lse_p, T_c = sum_{t_p=c}
pred[c, p], and w_c = 0.001 / (1 - 0.999**counts_c) (0 for empty classes):

    loss = (sum_c w_c * (S_c - T_c)) / (sum_c w_c * counts_c)

The histogram over the integer target is computed on the host (np.bincount
while laying out/sharding the inputs); the weights w_c then multiply the
per-class DEVICE partials, so the device never needs a counts pass:

  - fast path (all pixels valid, all w_c equal -- always true for this
    data regime, where 0.999**counts underflows and every w_c == 0.001):
    numerator = w * (sum_p lse_p - sum_c T_c), denominator = w * N.
    sum_p lse_p rides the ln activation's accum_out for free.
  - general path (ignore_index pixels or unequal weights): a per-pixel
    weight map W_p = w_(t_p) * valid_p is prepared on the host, and the
    device computes sum_p W_p*lse_p with one extra tensor_tensor per chunk.

Device pass per core (one batch; pixels on partitions, [128, 19, F] chunks):
  exp (one ACT instr over all 19 classes) -> sumexp over classes (DVE
  tensor_tensor chain, optional gpsimd side-chain) -> ln with accum_out
  (ACT) -> per class one fused STT (t==c)*pred_c with accum_out -> DMA out
  the [128, 19*NCH] fp32 T partials + [128, NCH] S partials.
"""

import os

import numpy as np

NCLASS = 19
B, H, W = 8, 512, 512
NPIX = H * W          # 262144 pixels per batch
P = 128               # SBUF partitions
FW = NPIX // P        # 2048 free-dim elements per partition
NCORES = 8
BETA = 1.0 - 0.001

F = int(os.environ.get("CHUNK_F", "1024"))  # free-dim chunk size
NCH = FW // F                               # chunks per batch
GPS_SUMEXP = int(os.environ.get("GPS_SUMEXP", "0"))  # adds on gpsimd
SUMEXP_TREE = int(os.environ.get("SUMEXP_TREE", "0"))
SKIP_STT = int(os.environ.get("SKIP_STT", "0"))      # attribution probes
SKIP_SUMEXP = int(os.environ.get("SKIP_SUMEXP", "0"))
SKIP_EXP = int(os.environ.get("SKIP_EXP", "0"))
# T-side select implementation:
#   stt      = scalar_tensor_tensor fused (1x-only uop, slow)
#   ts_ttr   = tensor_scalar mask @4x + tensor_tensor_reduce accum
#   ts_tt_pe = tensor_scalar mask @4x + tensor_tensor mult @2x + PE reduce
#   gps_tt_pe= gpsimd mask + tensor_tensor mult @2x + PE reduce
TSEL = os.environ.get("TSEL", "ts_tt_pe")
MMW = int(os.environ.get("MMW", "512"))

_COMPILED = {}


def _np_bf16():
    import ml_dtypes

    return ml_dtypes.bfloat16


def _patch_tile_drain():
    """walrus in this container rejects >1 sem-wait on one instruction
    ("Too many sync wait commands"); the tile-exit Drain carries one wait
    per logical processor. Split them into single-wait NOPs."""
    import bass_rust
    import concourse.tile as tile

    if getattr(tile.TileContext, "_drain_patched", False):
        return

    def _drain_and_barrier(self, tick_clock, wait_clock):
        from concourse.tile import ScopedClock

        probe = self.nc.sync.nop(nofuse=True)
        wait_clock.add_sem_waits(
            probe.ins, ScopedClock({None: tick_clock.global_clock})
        )
        si = probe.ins.sync_info
        waits = list(si.on_wait) if si else []
        if si:
            si.on_wait = waits[:1]
        for i in range(1, len(waits)):
            n = self.nc.sync.nop(nofuse=True)
            n.ins.sync_info = bass_rust.SyncInfo(
                on_wait=waits[i : i + 1], on_update=[]
            )
        self.nc.sync.drain()
        self.nc.all_engine_barrier()
        assert self.sems is not None
        popped = self.nc._tile_sem_poison_stack.pop()
        assert popped is self._sem_poison
        self.nc.clear_and_free_semaphores(list(self.sems.allocated().values()))
        self.nc.all_engine_barrier()

    tile.TileContext._drain_and_barrier = _drain_and_barrier
    tile.TileContext._drain_patched = True


def _split_excess_waits(nc, maxw=1):
    """Post-pass: any instruction carrying more than `maxw` sem-waits gets
    the extras moved onto same-engine NOPs inserted right before it (the
    engine executes in order, so semantics are identical)."""
    import bass_rust

    for blk in nc.m.functions[0].blocks:
        insts = list(blk.instructions)
        out = []
        changed = False
        for inst in insts:
            si = inst.sync_info
            if si is not None and si.on_wait and len(si.on_wait) > maxw:
                waits = list(si.on_wait)
                si.on_wait = waits[:maxw]
                extra = waits[maxw:]
                eng = nc.engines[inst.engine]
                for i in range(0, len(extra), maxw):
                    n = eng.nop(nofuse=True)
                    cur = nc.cur_bb.bb
                    cur_insts = list(cur.instructions)
                    assert cur_insts[-1].name == n.ins.name
                    cur.instructions = cur_insts[:-1]
                    n.ins.sync_info = bass_rust.SyncInfo(
                        on_wait=extra[i : i + maxw], on_update=[]
                    )
                    out.append(n.ins)
                changed = True
            out.append(inst)
        if changed:
            blk.instructions = out


def build_nc(reps: int = 1, general: bool = False):
    """Per-core Bass program (SPMD over 8 cores, one batch each).

    Inputs: pred [P, NCH, NCLASS, F] bf16 (class-major chunks, host
    pre-transposed), targ [P, FW] bf16 (class ids as floats; invalid
    pixels remapped to -5 so no is_equal matches). General variant adds
    wmap [P, FW] bf16 (per-pixel class weight, 0 for invalid).

    Outputs: out_t [P, NCH*NCLASS] fp32 (T partials, col k*19+c),
    out_s [P, NCH] fp32 (sum of lse per chunk; general: W-weighted).
    """
    from contextlib import ExitStack

    import concourse.bass as bass
    import concourse.tile as tile
    from concourse import mybir

    _patch_tile_drain()

    io_dt = mybir.dt.bfloat16
    nc = bass.Bass()
    pred = nc.declare_dram_parameter(
        "pred", [P, NCH, NCLASS, F], io_dt, isOutput=False
    )
    targ = nc.declare_dram_parameter("targ", [P, FW], io_dt, isOutput=False)
    if general:
        wmap = nc.declare_dram_parameter(
            "wmap", [P, FW], io_dt, isOutput=False
        )
    use_pe_out = TSEL in ("ts_tt_pe", "gps_tt_pe")
    if use_pe_out:
        out_tr = nc.declare_dram_parameter(
            "out_tr", [NCLASS, 1], mybir.dt.float32, isOutput=True
        )
    else:
        out_t = nc.declare_dram_parameter(
            "out_t", [P, NCH * NCLASS], mybir.dt.float32, isOutput=True
        )
    out_s = nc.declare_dram_parameter(
        "out_s", [P, NCH], mybir.dt.float32, isOutput=True
    )

    use_pe = TSEL in ("ts_tt_pe", "gps_tt_pe")

    with tile.TileContext(nc) as tc:
        with ExitStack() as ctx:
            io = ctx.enter_context(tc.tile_pool(name="io", bufs=2))
            work = ctx.enter_context(tc.tile_pool(name="work", bufs=2))
            pp = ctx.enter_context(tc.tile_pool(name="pp", bufs=3))
            acc = ctx.enter_context(tc.tile_pool(name="acc", bufs=1))
            if use_pe:
                psp = ctx.enter_context(
                    tc.tile_pool(name="psp", bufs=1, space="PSUM")
                )

            t_acc = acc.tile([P, NCH * NCLASS], mybir.dt.float32)
            nc.vector.memset(t_acc[:, :], 0.0)
            s_acc = acc.tile([P, NCH], mybir.dt.float32)
            nc.vector.memset(s_acc[:, :], 0.0)

            if use_pe:
                # place[:, c, :]: [128, NCLASS] one-hot col c -- stationary
                # operand routing a partition-sum into PSUM row c
                place = acc.tile([P, NCLASS, NCLASS], io_dt)
                nc.vector.memset(place[:, :, :], 0.0)
                for c in range(NCLASS):
                    nc.vector.memset(place[:, c, c : c + 1], 1.0)
                ps_t = psp.tile([NCLASS, F], mybir.dt.float32, tag="psT")
                t_red = acc.tile([NCLASS, 1], mybir.dt.float32)
                nc.vector.memset(t_red[:, :], 0.0)

            def _body():
                # Software pipeline, explicit per-engine program order:
                #   ACT: exp(0), exp(1), ..., ln(0), ln(1), ...
                #   DVE: STT(0), STT(1), ..., sumexp(0), sumexp(1), ...
                # so exp(k+1) never waits behind ln(k), and the STTs (which
                # only need the DMA) fill the DVE while ACT runs exp.
                p_tiles, t_tiles, e_tiles, w_tiles = [], [], [], []
                for k in range(NCH):
                    p_tile = io.tile([P, NCLASS, F], io_dt, tag="p")
                    nc.sync.dma_start(out=p_tile[:, :, :], in_=pred[:, k, :, :])
                    t_tile = io.tile([P, F], io_dt, tag="t")
                    nc.sync.dma_start(
                        out=t_tile[:], in_=targ[:, k * F : (k + 1) * F]
                    )
                    if general:
                        w_tile = io.tile([P, F], io_dt, tag="w")
                        nc.sync.dma_start(
                            out=w_tile[:], in_=wmap[:, k * F : (k + 1) * F]
                        )
                        w_tiles.append(w_tile)
                    p_tiles.append(p_tile)
                    t_tiles.append(t_tile)

                    # exp of all classes in one ACT instruction
                    if SKIP_EXP:
                        e_tiles.append(p_tile)  # probe: no ACT exp at all
                    else:
                        e_tile = work.tile([P, NCLASS, F], io_dt, tag="e")
                        nc.scalar.activation(
                            out=e_tile[:, :, :],
                            in_=p_tile[:, :, :],
                            func=mybir.ActivationFunctionType.Exp,
                        )
                        e_tiles.append(e_tile)

                    # T partials: per class, (t==c)*pred_c summed over pixels.
                    # scalar_tensor_tensor only has a 1x uop, so the fast
                    # variants split it: mask via tensor_scalar (4x) or
                    # gpsimd, product via tensor_tensor (2x), reduction via
                    # idle-PE matmul against the one-hot `place` stationary.
                    for c in range(NCLASS if not SKIP_STT else 0):
                        col = k * NCLASS + c
                        if TSEL == "stt":
                            prod = pp.tile([P, F], io_dt, tag="prod")
                            nc.vector.scalar_tensor_tensor(
                                out=prod[:],
                                in0=t_tile[:],
                                scalar=float(c),
                                in1=p_tile[:, c, :],
                                op0=mybir.AluOpType.is_equal,
                                op1=mybir.AluOpType.mult,
                                accum_out=t_acc[:, col : col + 1],
                            )
                            continue
                        mask = pp.tile([P, F], io_dt, tag="mask")
                        if TSEL == "gps_tt_pe":
                            nc.gpsimd.tensor_scalar(
                                out=mask[:],
                                in0=t_tile[:],
                                scalar1=float(c),
                                scalar2=None,
                                op0=mybir.AluOpType.is_equal,
                            )
                        else:
                            nc.vector.tensor_scalar(
                                out=mask[:],
                                in0=t_tile[:],
                                scalar1=float(c),
                                scalar2=None,
                                op0=mybir.AluOpType.is_equal,
                            )
                        if TSEL == "ts_ttr":
                            prod = pp.tile([P, F], io_dt, tag="prod")
                            nc.vector.tensor_tensor_reduce(
                                out=prod[:],
                                in0=mask[:],
                                in1=p_tile[:, c, :],
                                scale=1.0,
                                scalar=0.0,
                                op0=mybir.AluOpType.mult,
                                op1=mybir.AluOpType.add,
                                accum_out=t_acc[:, col : col + 1],
                            )
                        else:
                            prod = pp.tile([P, F], io_dt, tag="prod")
                            nc.vector.tensor_tensor(
                                prod[:], mask[:], p_tile[:, c, :],
                                mybir.AluOpType.mult,
                            )
                            for s in range(F // MMW):
                                nc.tensor.matmul(
                                    ps_t[:, s * MMW : (s + 1) * MMW],
                                    place[:, c, :],
                                    prod[:, s * MMW : (s + 1) * MMW],
                                    start=(k == 0 and c == 0),
                                    stop=(k == NCH - 1 and c == NCLASS - 1),
                                )

                # sumexp over classes, then ln (ACT) behind all exps
                sxs = []
                for k in range(NCH):
                    e_tile = e_tiles[k]
                    sx = work.tile([P, F], io_dt, tag="sx")
                    if SKIP_SUMEXP:
                        nc.vector.tensor_tensor(
                            sx[:], e_tile[:, 0, :], e_tile[:, 1, :],
                            mybir.AluOpType.add,
                        )
                    else:
                        nc.vector.tensor_tensor(
                            sx[:], e_tile[:, 0, :], e_tile[:, 1, :],
                            mybir.AluOpType.add,
                        )
                        for c in range(2, NCLASS):
                            nc.vector.tensor_tensor(
                                sx[:], sx[:], e_tile[:, c, :],
                                mybir.AluOpType.add,
                            )
                    sxs.append(sx)

                if use_pe and not SKIP_STT:
                    # drain PSUM: free-reduce [19, F] -> [19, 1]
                    junk = work.tile([NCLASS, F], io_dt, tag="junk")
                    nc.scalar.activation(
                        out=junk[:, :],
                        in_=ps_t[:, :],
                        func=mybir.ActivationFunctionType.Copy,
                        accum_out=t_red[:, 0:1],
                    )

                for k in range(NCH):
                    lse = work.tile([P, F], io_dt, tag="lse")
                    if general:
                        nc.scalar.activation(
                            out=lse[:],
                            in_=sxs[k][:],
                            func=mybir.ActivationFunctionType.Ln,
                        )
                        wl = pp.tile([P, F], io_dt, tag="wl")
                        nc.vector.tensor_tensor_reduce(
                            out=wl[:],
                            in0=w_tiles[k][:],
                            in1=lse[:],
                            scale=1.0,
                            scalar=0.0,
                            op0=mybir.AluOpType.mult,
                            op1=mybir.AluOpType.add,
                            accum_out=s_acc[:, k : k + 1],
                        )
                    else:
                        # sum_f lse rides the ln's accum_out for free
                        nc.scalar.activation(
                            out=lse[:],
                            in_=sxs[k][:],
                            func=mybir.ActivationFunctionType.Ln,
                            accum_out=s_acc[:, k : k + 1],
                        )

            if reps == 1:
                _body()
            else:
                with tc.For_i(0, reps, 1):
                    _body()

            if use_pe:
                nc.sync.dma_start(out=out_tr[:, :], in_=t_red[:, :])
            else:
                nc.sync.dma_start(out=out_t[:, :], in_=t_acc[:, :])
            nc.sync.dma_start(out=out_s[:, :], in_=s_acc[:, :])

    _split_excess_waits(nc, maxw=1)
    return nc


V4 = int(os.environ.get("V4", "1"))


def get_nc(reps: int = 1, general: bool = False):
    key = (reps, general, V4)
    if key not in _COMPILED:
        if V4:
            import concourse.bass as bass
            import concourse.tile as tile
            from concourse import mybir

            import kernel_v4

            _COMPILED[key] = kernel_v4.build(
                None, bass, tile, mybir, reps, general,
                _patch_tile_drain, _split_excess_waits,
            )
        else:
            _COMPILED[key] = build_nc(reps, general)
    return _COMPILED[key]


def _class_weights(targ_np):
    """Histogram of the full target -> quantized class weights (bf16, so
    the device W map and the host denominator use identical values)."""
    valid = targ_np >= 0
    counts = np.bincount(
        targ_np[valid].ravel().astype(np.int64), minlength=NCLASS
    )[:NCLASS].astype(np.float64)
    with np.errstate(divide="ignore", over="ignore", under="ignore"):
        w = np.float32(1.0 - BETA) / (
            1.0 - np.float32(BETA) ** counts.astype(np.float32)
        )
    w = np.where(counts > 0, w, 0.0).astype(np.float32)
    wq = w.astype(_np_bf16()).astype(np.float64)
    return counts, wq, valid


def _shard_inputs(pred_np, targ_np, wmap=None, t_enc=None):
    dt = _np_bf16()
    if t_enc is None:
        t_enc = targ_np.astype(np.float32)
    in_maps = []
    for b in range(NCORES):
        if V4:
            # [19, 262144] -> [P, NCLASS, FW]
            pb = pred_np[b].reshape(NCLASS, P, FW).transpose(1, 0, 2)
        else:
            # [19, 262144] -> [P, NCH, NCLASS, F]
            pb = pred_np[b].reshape(NCLASS, P, NCH, F).transpose(1, 2, 0, 3)
        m = {
            "pred": np.ascontiguousarray(pb).astype(dt),
            "targ": t_enc[b].reshape(P, FW).astype(dt),
        }
        if wmap is not None:
            m["wmap"] = wmap[b].reshape(P, FW).astype(dt)
        in_maps.append(m)
    return in_maps


def _run_device(pred_np, targ_np, reps: int = 1, in_maps=None, general=False):
    from concourse.bass_utils import run_bass_kernel_spmd

    nc = get_nc(reps, general)
    if in_maps is None:
        in_maps = _shard_inputs(pred_np, targ_np)
    res = run_bass_kernel_spmd(nc, in_maps, core_ids=list(range(NCORES)))
    return [res.results[i] for i in range(NCORES)]


def kernel(pred: np.ndarray, target: np.ndarray) -> np.ndarray:
    pred_np = np.asarray(pred, dtype=np.float32)
    targ_np = np.asarray(target)

    counts, wq, valid = _class_weights(targ_np)
    uniform = bool(valid.all()) and bool(
        np.all(wq == wq[0]) and wq[0] > 0
    )

    if uniform:
        in_maps = _shard_inputs(pred_np, targ_np)
        outs = _run_device(pred_np, targ_np, in_maps=in_maps, general=False)
        S = np.float64(0.0)
        T = np.zeros(NCLASS, np.float64)
        for r in outs:
            T += _t_partials(r)
            S += np.asarray(r["out_s"], np.float64).sum()
        num = wq[0] * (S - float(T.sum()))
        den = wq[0] * float(counts.sum())
        return np.array(np.float32(num / den))

    # general path: per-pixel weight map; invalid pixels get weight 0 and
    # a target code (-5) that matches no class
    wmap = (wq[np.clip(targ_np, 0, NCLASS - 1)] * valid).astype(np.float32)
    t_enc = np.where(valid, targ_np, -5).astype(np.float32)
    in_maps = _shard_inputs(pred_np, targ_np, wmap=wmap, t_enc=t_enc)
    outs = _run_device(pred_np, targ_np, in_maps=in_maps, general=True)
    SW = np.float64(0.0)
    T = np.zeros(NCLASS, np.float64)
    for r in outs:
        T += _t_partials(r)
        SW += np.asarray(r["out_s"], np.float64).sum()
    num = SW - float((wq * T).sum())
    den = float((wq * counts).sum())
    return np.array(np.float32(num / den))


def _t_partials(r):
    if "out_tr" in r:
        return np.asarray(r["out_tr"], np.float64)[:, 0]
    return (
        np.asarray(r["out_t"], np.float64).reshape(P, NCH, NCLASS).sum((0, 1))
    )
